# revision 1
# baseline (speedup 1.0000x reference)
"""FBGCN layer kernel for 8 Trainium2 NeuronCores.

out = aL * GCNConv(x, edge_index; W_conv, b_conv) + aH * (Lsym @ relu(x @ W_high.T))

Sharding: 1D row-partition of output nodes across 8 cores (1536 rows each).
Each core:
  - computes Y = relu(x @ W_high.T) and xw = x @ W_conv.T for ALL nodes
    (x is replicated; this is a tiny matmul), writes xw to a DRAM scratch,
  - streams its column slice of aH*Lsym.T (fp16) through the PE with Y
    blocks stationary, accumulating HhT = (aH*Lsym_rows @ Y).T in PSUM,
  - gathers per-edge source rows of xw from the scratch with dma_gather
    (edges pre-sorted by target on the host), multiplies by a host-built
    sparse "segment matrix" (norm weights folded in) on the PE to produce
    the GCN aggregation per 128-target block,
  - transposes HhT blocks on the PE and adds, writing out[1536, 64] fp32.
No cross-core communication is needed.
"""

import numpy as np

import concourse.bacc as bacc
import concourse.mybir as mybir
import concourse.tile as tile
from concourse.bass_utils import run_bass_kernel_spmd

N, E, D = 12288, 196608, 64
NCORES = 8
M = N // NCORES          # 1536 output rows per core
MB = M // 128            # 12 target blocks per core
KB = N // 128            # 96 contraction blocks
F2 = 2 * D               # 128 (feature dim padded for 256B gather rows)
BIAS_ROW = N             # scratch row holding b_conv
ZERO_ROW = N + 1         # scratch row of zeros (dummy gather target)
SCR_ROWS = N + 2
XT_CHUNK = 24            # kb blocks per xT DMA chunk
LS_PACK = 4              # lsymT kb tiles per DMA

DT = mybir.dt.float16
NPDT = np.float16
F32 = mybir.dt.float32
AFT = mybir.ActivationFunctionType


def _build_program(C: int):
    """Build the SPMD Bass program. C = edge chunks (of 128 slots) per
    128-target block; S = total edge slots per core."""
    S = MB * C * 128
    nc = bacc.Bacc("TRN2", target_bir_lowering=False, debug=False,
                   num_devices=NCORES)

    lsymT = nc.dram_tensor("lsymT", [N, M], DT, kind="ExternalInput")
    xT = nc.dram_tensor("xT", [D, N], F32, kind="ExternalInput")
    wt2 = nc.dram_tensor("wt2", [D, F2], F32, kind="ExternalInput")
    segT = nc.dram_tensor("segT", [S, 128], DT, kind="ExternalInput")
    gidx = nc.dram_tensor("gidx", [128, S // 16], mybir.dt.int16,
                          kind="ExternalInput")
    bvec = nc.dram_tensor("bvec", [1, F2], DT, kind="ExternalInput")
    ident = nc.dram_tensor("ident", [D, D], F32, kind="ExternalInput")
    outp = nc.dram_tensor("out", [M, D], F32, kind="ExternalOutput")

    with tile.TileContext(nc) as tc:
        with (
            tc.tile_pool(name="consts", bufs=1) as consts,
            tc.tile_pool(name="dram", bufs=1, space="DRAM") as dram,
            tc.tile_pool(name="xt", bufs=2) as xt_pool,
            tc.tile_pool(name="xw", bufs=4) as xw_pool,
            tc.tile_pool(name="ls", bufs=3) as ls_pool,
            tc.tile_pool(name="seg", bufs=2) as seg_pool,
            tc.tile_pool(name="msg", bufs=2) as msg_pool,
            tc.tile_pool(name="ob", bufs=3) as ob_pool,
            tc.tile_pool(name="psb", bufs=1, space="PSUM") as ps_big,
            tc.tile_pool(name="pss", bufs=4, space="PSUM") as ps_small,
        ):
            # ---- constants / persistent tiles ----
            wt2_sb = consts.tile([D, F2], F32, tag="wt2")
            nc.sync.dma_start(wt2_sb[:], wt2[:])
            ident_sb = consts.tile([D, D], F32, tag="ident")
            nc.sync.dma_start(ident_sb[:], ident[:])
            bvec_sb = consts.tile([1, F2], DT, tag="bvec")
            nc.sync.dma_start(bvec_sb[:], bvec[:])
            idx_sb = consts.tile([128, S // 16], mybir.dt.int16, tag="idx")
            nc.sync.dma_start(idx_sb[:], gidx[:])
            zeros_sb = consts.tile([128, 1024], DT, tag="zeros")
            nc.vector.memset(zeros_sb[:], 0)
            y_all = consts.tile([128, KB * D], DT, tag="yall")
            hh_sb = consts.tile([D, M], F32, tag="hh")

            scratch = dram.tile([SCR_ROWS, F2], DT, tag="scr")

            # ---- zero-fill scratch, then bias row ----
            for r0 in range(0, N, 1024):
                nc.sync.dma_start(
                    scratch[r0:r0 + 1024, :].rearrange("(a p) f -> p a f", p=128),
                    zeros_sb[:].rearrange("p (a f) -> p a f", f=F2),
                )
            nc.sync.dma_start(scratch[N:N + 2, :], zeros_sb[0:2, 0:F2])
            nc.sync.dma_start(scratch[BIAS_ROW:BIAS_ROW + 1, :], bvec_sb[:])

            # ---- phase A0: Y = relu(x@Wh.T), xw = x@Wc.T for all nodes ----
            xt_sb = None
            for kb in range(KB):
                if kb % XT_CHUNK == 0:
                    xt_sb = xt_pool.tile([D, XT_CHUNK * 128], F32, tag="xt")
                    c0 = kb * 128
                    nc.sync.dma_start(xt_sb[:], xT[:, c0:c0 + XT_CHUNK * 128])
                ps = ps_small.tile([128, F2], F32, tag="ps")
                nc.tensor.matmul(
                    ps[:],
                    lhsT=xt_sb[:, (kb % XT_CHUNK) * 128:(kb % XT_CHUNK + 1) * 128],
                    rhs=wt2_sb[:],
                    start=True, stop=True,
                )
                nc.scalar.activation(y_all[:, kb * D:(kb + 1) * D], ps[:, 0:D],
                                     AFT.Relu)
                xw_sb = xw_pool.tile([128, D], DT, tag="xw")
                nc.scalar.activation(xw_sb[:], ps[:, D:F2], AFT.Copy)
                nc.sync.dma_start(scratch[kb * 128:(kb + 1) * 128, 0:D], xw_sb[:])

            # ---- phase A1: HhT += Y_kb.T @ lsymT_kb over all kb ----
            hhps = ps_big.tile([D, M], F32, tag="hh")
            for kb in range(KB):
                if kb % LS_PACK == 0:
                    ls_sb = ls_pool.tile([128, LS_PACK * M], DT, tag="ls")
                    r0 = kb * 128
                    nc.sync.dma_start(
                        ls_sb[:].rearrange("p (t m) -> p t m", t=LS_PACK),
                        lsymT[r0:r0 + LS_PACK * 128, :]
                        .rearrange("(t p) m -> p t m", p=128),
                    )
                lsv = ls_sb[:].rearrange("p (t m) -> p t m", t=LS_PACK)
                for mc in range(M // 512):
                    nc.tensor.matmul(
                        hhps[:, mc * 512:(mc + 1) * 512],
                        lhsT=y_all[:, kb * D:(kb + 1) * D],
                        rhs=lsv[:, kb % LS_PACK, mc * 512:(mc + 1) * 512],
                        start=(kb == 0), stop=(kb == KB - 1),
                    )
            nc.vector.tensor_copy(hh_sb[:], hhps[:])

            # ---- phase C: GCN aggregation per 128-target block + combine ----
            for b in range(MB):
                seg_sb = seg_pool.tile([128, C * 128], DT, tag="seg")
                s0 = b * C * 128
                nc.sync.dma_start(
                    seg_sb[:].rearrange("p (c t) -> p c t", c=C),
                    segT[s0:s0 + C * 128, :].rearrange("(c p) t -> p c t", p=128),
                )
                msg_sb = msg_pool.tile([128, C * F2], DT, tag="msg")
                nc.gpsimd.dma_gather(
                    msg_sb[:].rearrange("p (c f) -> p c f", c=C),
                    scratch[:],
                    idx_sb[:, b * C * 8:(b + 1) * C * 8],
                    C * 128, C * 128, F2,
                )
                segv = seg_sb[:].rearrange("p (c t) -> p c t", c=C)
                msgv = msg_sb[:].rearrange("p (c f) -> p c f", c=C)
                hl = ps_small.tile([128, F2], F32, tag="ps")
                for c in range(C):
                    nc.tensor.matmul(
                        hl[:],
                        lhsT=segv[:, c, :],
                        rhs=msgv[:, c, :],
                        start=(c == 0), stop=(c == C - 1),
                    )
                pt = ps_small.tile([128, D], F32, tag="ps")
                nc.tensor.transpose(pt[:], hh_sb[:, b * 128:(b + 1) * 128],
                                    ident_sb[:])
                ob = ob_pool.tile([128, D], F32, tag="ob")
                nc.vector.tensor_add(ob[:], hl[:, 0:D], pt[:])
                nc.sync.dma_start(outp[b * 128:(b + 1) * 128, :], ob[:])

    nc.compile()
    return nc


def _prepare_host(x, edge_index, Lsym, W_high, W_conv, b_conv, aL, aH):
    """Shard + preprocess inputs. Returns (in_maps, C)."""
    x = np.asarray(x, np.float32)
    edge_index = np.asarray(edge_index)
    Lsym = np.asarray(Lsym, np.float32)
    W_high = np.asarray(W_high, np.float32)
    W_conv = np.asarray(W_conv, np.float32)
    b_conv = np.asarray(b_conv, np.float32)
    aL = float(np.asarray(aL))
    aH = float(np.asarray(aH))

    src_e = edge_index[0].astype(np.int64)
    tgt_e = edge_index[1].astype(np.int64)

    # degrees with self loops (matches PyG GCNConv gcn_norm)
    deg = np.bincount(tgt_e, minlength=N).astype(np.float64) + 1.0
    dinv = 1.0 / np.sqrt(deg)

    # full edge list: graph edges + self loops + one bias edge per target
    loops = np.arange(N, dtype=np.int64)
    srcs = np.concatenate([src_e, loops, np.full(N, BIAS_ROW, np.int64)])
    tgts = np.concatenate([tgt_e, loops, loops])
    w = np.concatenate([
        aL * dinv[src_e] * dinv[tgt_e],
        aL * dinv * dinv,
        np.full(N, aL, np.float64),
    ]).astype(np.float32)

    order = np.argsort(tgts, kind="stable")
    srcs, tgts, w = srcs[order], tgts[order], w[order]

    # group = global 128-target block id (0..95); sorted order groups them
    grp = tgts // 128
    counts = np.bincount(grp, minlength=KB)
    C = int(np.ceil(counts.max() / 128))
    S = MB * C * 128

    grp_start = np.zeros(KB, np.int64)
    grp_start[1:] = np.cumsum(counts)[:-1]
    pos = np.arange(len(tgts)) - grp_start[grp]
    core = grp // MB
    slot = (grp % MB) * C * 128 + pos  # slot within the core's edge array

    gidx_all = np.full((NCORES, S), ZERO_ROW, np.int16)
    segT_all = np.zeros((NCORES, S, 128), NPDT)
    gidx_all[core, slot] = srcs.astype(np.int16)
    segT_all[core, slot, tgts % 128] = w.astype(NPDT)

    xT = np.ascontiguousarray(x.T)
    wt2 = np.ascontiguousarray(
        np.concatenate([W_high.T, W_conv.T], axis=1), dtype=np.float32)
    bvec = np.zeros((1, F2), NPDT)
    bvec[0, :D] = b_conv.astype(NPDT)
    ident = np.eye(D, dtype=np.float32)

    in_maps = []
    for j in range(NCORES):
        lsymT_j = np.ascontiguousarray(
            (aH * Lsym[j * M:(j + 1) * M, :]).T.astype(NPDT))
        g = gidx_all[j]
        gw = np.ascontiguousarray(g.reshape(S // 16, 16).T)  # [16, S/16]
        in_maps.append({
            "lsymT": lsymT_j,
            "xT": xT,
            "wt2": wt2,
            "segT": np.ascontiguousarray(segT_all[j]),
            "gidx": np.ascontiguousarray(np.tile(gw, (8, 1))),
            "bvec": bvec,
            "ident": ident,
        })
    return in_maps, C


_CACHE = {}


def kernel(x, edge_index, Lsym, W_high, W_conv, b_conv, aL, aH):
    in_maps, C = _prepare_host(x, edge_index, Lsym, W_high, W_conv, b_conv,
                               aL, aH)
    nc = _CACHE.get(C)
    if nc is None:
        nc = _build_program(C)
        _CACHE[C] = nc
    res = run_bass_kernel_spmd(nc, in_maps, core_ids=list(range(NCORES)))
    out = np.concatenate([res.results[j]["out"] for j in range(NCORES)], axis=0)
    return out.astype(np.float32)
